# revision 1
# baseline (speedup 1.0000x reference)
"""Distributed Bass kernel for sparse cluster attention on 8 TRN2 NeuronCores.

Token-sharded inputs (minimal host->device bytes), head-parallel compute:
  0. AllGather weight shards (2 MB/core) -> full W on every core.
  A. qkv for this core's 2048-token slice, all 16 heads; AllToAll
     redistributes to head-sharded layout (qT [128ch, N]; k,v [tok, ch]).
  B. fp32-accurate keyframe q*k (hi/lo bf16 split) for this core's 256
     keyframe tokens over all heads, max over heads -> AllGather scores.
  3. on-device top-153 per cluster via rank comparison; packed global
     token-id rows give one-hot windows.
  3b. gather k (-> [ch, j]) and v (-> packed [j, ch]) via one-hot matmuls.
  4. flash-style attention per consumer cluster (logits MM -> exp on ACT ->
     AV MM with ones-augmented v); per-src prefix chunking over packed kv.
  5. AllToAll of attention output -> proj on this core's token slice ->
     out [2048, 1024] f32; host concatenates.
"""

import numpy as np
import ml_dtypes

import os
import concourse.bass as bass
import concourse.bacc as bacc
import concourse.mybir as mybir
import concourse.tile as tile
from concourse.bass_utils import run_bass_kernel_spmd

BF16 = mybir.dt.bfloat16
F32 = mybir.dt.float32
I32 = mybir.dt.int32
AF = mybir.ActivationFunctionType
OP = mybir.AluOpType

# problem constants
H, D, C = 16, 64, 1024
S, P = 32, 512
K, FC = 4, 8
N = S * P                      # 16384 tokens
TK = 153                       # top-k patches per cluster
NSUB = 5                       # subsampled frames
NCORES = 8
HC = H // NCORES               # heads per core = 2
CHC = HC * D                   # channels per core = 128
TOKS = N // NCORES             # tokens per core slice = 2048
KFC = K * P // NCORES          # keyframe tokens per core = 256
SCALE = float(D) ** -0.5
KFT = K * P                    # keyframe tokens = 2048
FULL = FC * TK                 # packed kv rows per full src block = 1224
PRE5 = NSUB * TK               # packed kv rows per 5-frame prefix = 765
KGW = 1280                     # kg tile width (>= FULL, mult of 128)

_CACHE: dict = {}


def _chunks_for(ci):
    """(src, chunk, rows) list for consumer cluster ci over packed kv."""
    out = []
    for src in range(K):
        valid = FULL if src in (0, ci) else PRE5
        nch = (valid + 127) // 128
        for c in range(nch):
            out.append((src, c, min(128, valid - c * 128)))
    return out


def _win_frames(c):
    """Frames whose packed rows [f*153, (f+1)*153) intersect window
    [128c, 128(c+1))."""
    lo, hi = 128 * c, 128 * (c + 1)
    return [f for f in range(FC) if f * TK < hi and (f + 1) * TK > lo]


def build_nc(clusters, keyframes):
    STUB = os.environ.get("KSTUB", "0") == "1"
    KCUT = int(os.environ.get("KCUT", "9"))
    nc = bacc.Bacc(None, target_bir_lowering=False, debug=False)

    # ---- kernel I/O (token-sharded; host preps per-core slices) ----
    xsT = nc.dram_tensor("xsT", [C, TOKS], BF16, kind="ExternalInput")
    xkfT_h = nc.dram_tensor("xkfT_h", [C, KFC], BF16, kind="ExternalInput")
    xkfT_l = nc.dram_tensor("xkfT_l", [C, KFC], BF16, kind="ExternalInput")
    # shard cols: 0:384 qkv | 384:512 q hi | 512:640 k hi | 640:768 q lo |
    #             768:896 k lo | 896:1024 proj
    wcat = nc.dram_tensor("wcat", [C, C], BF16, kind="ExternalInput")
    bqkv = nc.dram_tensor("bqkv", [3 * C], F32, kind="ExternalInput")
    bproj = nc.dram_tensor("bproj", [C], F32, kind="ExternalInput")
    F16 = mybir.dt.float16
    out_ext = nc.dram_tensor("out", [TOKS, C], F16, kind="ExternalOutput")

    # ---- internal DRAM ----
    wg_in = nc.dram_tensor("wg_in", [C, C], BF16)
    wg_out = nc.dram_tensor("wg_out", [NCORES, C, C], BF16, addr_space="Shared")
    qkv_send = nc.dram_tensor("qkv_send", [NCORES, 384, TOKS], BF16)
    qkv_recv = nc.dram_tensor("qkv_recv", [NCORES, 384, TOKS], BF16)
    sc_in = nc.dram_tensor("sc_in", [KFC], F32)
    sc_out = nc.dram_tensor("sc_out", [K * P], F32, addr_space="Shared")
    ag_in = [nc.dram_tensor(f"ag_in{i}", [NCORES, CHC, TOKS // 2], BF16) for i in range(2)]
    ag_out = [nc.dram_tensor(f"ag_out{i}", [NCORES, CHC, TOKS // 2], BF16) for i in range(2)]

    def coll(kind, op, ins, outs):
        nc.gpsimd.collective_compute(
            kind, op, replica_groups=[list(range(NCORES))], ins=ins, outs=outs)

    if STUB:
        with tile.TileContext(nc) as tc:
            with tc.tile_pool(name="sp", bufs=2) as sp:
                t = sp.tile([128, 512], BF16)
                nc.sync.dma_start(t[:], xsT.ap()[0:128, 0:512])
                t2 = sp.tile([128, 512], F32)
                nc.vector.tensor_copy(t2[:], t[:])
                nc.sync.dma_start(out_ext.ap()[0:128, 0:512], t2[:])
        nc.finalize()
        return nc

    with tile.TileContext(nc) as tc:
        with (
            tc.tile_pool(name="persist", bufs=1) as pp,
            tc.tile_pool(name="work", bufs=3) as wp,
            tc.tile_pool(name="xp", bufs=8) as xp,
            tc.tile_pool(name="expw", bufs=2) as ep,
            tc.tile_pool(name="psmed", bufs=2, space="PSUM") as psM,
            tc.tile_pool(name="psav", bufs=2, space="PSUM") as psV,
            tc.tile_pool(name="psbig", bufs=1, space="PSUM") as psL,
        ):
            # ================= phase 0: AllGather weights =================
            nc.sync.dma_start(wg_in.ap(), wcat.ap())
            coll("AllGather", OP.bypass, [wg_in.ap().opt()], [wg_out.ap().opt()])

            # ================= persistent SBUF =================
            qT = pp.tile([CHC, N], BF16, tag="qT")                 # 4 MB
            k_sb = pp.tile([128, N // 128, CHC], BF16, tag="ksb")  # 4 MB
            v_sb = pp.tile([128, N // 128, CHC], BF16, tag="vsb")  # 4 MB
            kg = pp.tile([128, K, KGW], BF16, tag="kg")            # 1.25 MB
            vaug = pp.tile([128, K, 10, 130], BF16, tag="vaug")    # 1.3 MB
            ones_rowb = pp.tile([1, 128], BF16, tag="onesb")
            nc.vector.memset(ones_rowb[:], 1.0)
            onesf_row = pp.tile([1, 128], F32, tag="onesf")
            nc.vector.memset(onesf_row[:], 1.0)

            # full weights from the AllGather: [cc, q dst-major | kv dst-major].
            # Aliased onto the qT slot -- dead before qT is written (post-A2A).
            wqkv_all = pp.tile([128, 8, 3 * C], BF16, tag="qT", name="wqkv_all")
            for dst in range(NCORES):
                nc.sync.dma_start(
                    wqkv_all[:, :, dst * 128:(dst + 1) * 128],
                    wg_out.ap()[dst, :, 0:CHC].rearrange("(a p) c -> p a c", p=128))
                nc.sync.dma_start(
                    wqkv_all[:, :, C + dst * 256:C + (dst + 1) * 256],
                    wg_out.ap()[dst, :, CHC:3 * CHC].rearrange("(a p) c -> p a c", p=128))
            # keyframe-score weights [cc, q|k dst-major]; alias ksb/vsb slots.
            whi_qk = pp.tile([128, 8, 2 * C], BF16, tag="ksb", name="whi_qk")
            wlo_qk = pp.tile([128, 8, 2 * C], BF16, tag="vsb", name="wlo_qk")
            for dst in range(NCORES):
                for t_, qc0, kc0 in ((whi_qk, 384, 512), (wlo_qk, 640, 768)):
                    nc.sync.dma_start(
                        t_[:, :, dst * 128:(dst + 1) * 128],
                        wg_out.ap()[dst, :, qc0:qc0 + 128].rearrange("(a p) c -> p a c", p=128))
                    nc.sync.dma_start(
                        t_[:, :, C + dst * 128:C + (dst + 1) * 128],
                        wg_out.ap()[dst, :, kc0:kc0 + 128].rearrange("(a p) c -> p a c", p=128))

            # biases
            bq_all = pp.tile([128, 8], F32, tag="bqall")
            nc.sync.dma_start(bq_all[:], bqkv.ap()[0:C].rearrange("(d p) -> p d", p=128))
            bkv_b = pp.tile([1, 8, 2 * CHC], BF16, tag="bkvb")
            bkv_f = wp.tile([1, 8, 2 * CHC], F32, tag="qs2", bufs=1, name="bkv_f")
            nc.sync.dma_start(bkv_f[:, :, 0:CHC],
                              bqkv.ap()[C:2 * C].rearrange("(a d c) -> a d c", a=1, d=8))
            nc.sync.dma_start(bkv_f[:, :, CHC:2 * CHC],
                              bqkv.ap()[2 * C:3 * C].rearrange("(a d c) -> a d c", a=1, d=8))
            nc.vector.tensor_copy(bkv_b[:], bkv_f[:])
            bqk_rowb = pp.tile([1, 2 * C], BF16, tag="bqkrow")
            bqk_rowf = wp.tile([1, 2 * C], F32, tag="qs2", bufs=1, name="bqk_rowf")
            nc.sync.dma_start(bqk_rowf[:, 0:C], bqkv.ap()[0:C].rearrange("(a c) -> a c", a=1))
            nc.sync.dma_start(bqk_rowf[:, C:2 * C], bqkv.ap()[C:2 * C].rearrange("(a c) -> a c", a=1))
            nc.vector.tensor_copy(bqk_rowb[:], bqk_rowf[:])

            # ================= phase A: qkv for local tokens, all heads ======
            for tt in range(TOKS // 512 if KCUT >= 1 else 0):
                xt = [xp.tile([128, 512], BF16, tag="xmain", name=f"xt{tt}_{i}") for i in range(8)]
                for cc in range(8):
                    nc.sync.dma_start(xt[cc][:], xsT.ap()[cc * 128:(cc + 1) * 128, tt * 512:(tt + 1) * 512])
                for dst in range(NCORES):
                    psq = psM.tile([128, 512], F32, tag="med")
                    for cc in range(8):
                        nc.tensor.matmul(psq[:], wqkv_all[:, cc, dst * 128:(dst + 1) * 128],
                                         xt[cc][:], start=(cc == 0), stop=(cc == 7))
                    qsend = ep.tile([128, 512], BF16, tag="qsend", bufs=4,
                                    name=f"qsend{tt}_{dst}")
                    nc.vector.tensor_scalar(qsend[:], psq[:], bq_all[:, dst:dst + 1], None, OP.add)
                    nc.sync.dma_start(qkv_send.ap()[dst, 0:128, tt * 512:(tt + 1) * 512],
                                      qsend[:])
                for dp in range(NCORES // 2):
                    kvsend = ep.tile([128, 4, 512], BF16, tag="kvsend")
                    for sub in range(4):
                        pskv = psM.tile([128, 512], F32, tag="med")
                        for cc in range(8):
                            nc.tensor.matmul(pskv[:], xt[cc][:, sub * 128:(sub + 1) * 128],
                                             wqkv_all[:, cc, C + dp * 512:C + (dp + 1) * 512],
                                             start=(cc == 0), stop=False)
                        nc.tensor.matmul(pskv[:], ones_rowb[:],
                                         bkv_b[:, 2 * dp:2 * dp + 2, :].rearrange("a d c -> a (d c)"),
                                         start=False, stop=True)
                        nc.vector.tensor_copy(kvsend[:, sub, :], pskv[:])
                    for half in range(2):
                        dst = 2 * dp + half
                        nc.sync.dma_start(
                            qkv_send.ap()[dst, 128 + tt * 64:128 + (tt + 1) * 64, :]
                            .rearrange("r c -> (r c)")
                            .rearrange("(s p c) -> p s c", p=128, c=2 * CHC),
                            kvsend[:, :, half * 256:(half + 1) * 256])

            # ================= phase B: keyframe scores ======================
            for t2 in range(KFC // 128 if KCUT >= 1 else 0):
                xkh = [xp.tile([128, 128], BF16, tag="xkf", name=f"xkh{t2}_{i}", bufs=16) for i in range(8)]
                xkl = [xp.tile([128, 128], BF16, tag="xkf", name=f"xkl{t2}_{i}", bufs=16) for i in range(8)]
                for cc in range(8):
                    nc.sync.dma_start(xkh[cc][:], xkfT_h.ap()[cc * 128:(cc + 1) * 128, t2 * 128:(t2 + 1) * 128])
                    nc.sync.dma_start(xkl[cc][:], xkfT_l.ap()[cc * 128:(cc + 1) * 128, t2 * 128:(t2 + 1) * 128])
                psb2t = [psL.tile([128, 1024], F32, tag="lg", bufs=2, name=f"psb{t2}_{i}")
                         for i in range(2)]
                for pi, (w_, xk_) in enumerate(((whi_qk, xkh), (whi_qk, xkl), (wlo_qk, xkh))):
                    for cc in range(8):
                        first = (pi == 0 and cc == 0)
                        for half in range(4):
                            hsl = slice(half * 512, (half + 1) * 512)
                            nc.tensor.matmul(psb2t[half // 2][:, (half % 2) * 512:(half % 2 + 1) * 512],
                                             xk_[cc][:], w_[:, cc, hsl],
                                             start=first, stop=False)
                for half in range(4):
                    nc.tensor.matmul(psb2t[half // 2][:, (half % 2) * 512:(half % 2 + 1) * 512],
                                     ones_rowb[:], bqk_rowb[:, half * 512:(half + 1) * 512],
                                     start=False, stop=True)
                qs2 = wp.tile([128, 2 * C], F32, tag="qs2", bufs=1)
                nc.vector.tensor_copy(qs2[:, 0:C], psb2t[0][:])
                nc.vector.tensor_copy(qs2[:, C:2 * C], psb2t[1][:])
                nc.vector.tensor_tensor(qs2[:, 0:C], qs2[:, 0:C], qs2[:, C:2 * C], OP.mult)
                hs = wp.tile([128, 16], F32, tag="hs", bufs=1)
                for h in range(16):
                    nc.vector.reduce_sum(hs[:, h:h + 1], qs2[:, h * 64:(h + 1) * 64],
                                         axis=mybir.AxisListType.X)
                smax_c = wp.tile([128, 1], F32, tag="smaxc", bufs=2)
                nc.vector.reduce_max(smax_c[:], hs[:], axis=mybir.AxisListType.X)
                nc.sync.dma_start(
                    sc_in.ap()[t2 * 128:(t2 + 1) * 128].rearrange("(p a) -> p a", a=1),
                    smax_c[:])

            # ================= collectives: scores AG + qkv A2A ==============
            coll("AllToAll", OP.bypass, [qkv_send.ap().opt()], [qkv_recv.ap().opt()])
            coll("AllGather", OP.bypass, [sc_in.ap().opt()], [sc_out.ap().opt()])

            # ================= phase 3: top-k -> packed token-id rows ========
            iota160 = wp.tile([128, 160], I32, tag="io160", bufs=1)
            nc.gpsimd.iota(iota160[:], pattern=[[1, 160]], base=0, channel_multiplier=0)
            iota160f = pp.tile([128, 160], F32, tag="io160f")
            nc.vector.tensor_copy(iota160f[:], iota160[:])
            iota_pv = wp.tile([128, 4], I32, tag="iopv", bufs=1)
            nc.gpsimd.iota(iota_pv[:], pattern=[[128, 4]], base=0, channel_multiplier=1)
            iota_pvf = pp.tile([128, 4], F32, tag="iopvf")
            nc.vector.tensor_copy(iota_pvf[:], iota_pv[:])
            # global-token iota: iota_tc[p, tc] = 128*tc + p
            iota_tc = wp.tile([128, N // 128], I32, tag="iotc", bufs=1)
            nc.gpsimd.iota(iota_tc[:], pattern=[[128, N // 128]], base=0, channel_multiplier=1)
            iota_tcf = pp.tile([128, N // 128], F32, tag="iotcf")
            nc.vector.tensor_copy(iota_tcf[:], iota_tc[:])

            psel_rows = {}
            for cl in range(K if KCUT >= 2 else 0):
                s_row = wp.tile([1, P], F32, tag="srow", bufs=1)
                nc.sync.dma_start(s_row[:], sc_out.ap()[cl * P:(cl + 1) * P].rearrange("(a c) -> a c", a=1))
                s_colT = wp.tile([128, 4], F32, tag="scolT", bufs=1)
                nc.sync.dma_start(
                    s_colT[:], sc_out.ap()[cl * P:(cl + 1) * P].rearrange("(a p) -> p a", p=128))
                ps_bc = psM.tile([128, P], F32, tag="med")
                nc.tensor.matmul(ps_bc[:], onesf_row[:], s_row[:], start=True, stop=True)
                s_bc = wp.tile([128, P], F32, tag="sbc", bufs=1)
                nc.vector.tensor_copy(s_bc[:], ps_bc[:])
                ps_row = psM.tile([1, 160], F32, tag="med")
                for pc in range(4):
                    gt = wp.tile([128, P], BF16, tag="gtm", bufs=2)
                    nc.vector.tensor_scalar(gt[:], s_bc[:], s_colT[:, pc:pc + 1], None, OP.is_gt)
                    rank = wp.tile([128, 1], F32, tag="rank", bufs=2)
                    nc.vector.reduce_sum(rank[:], gt[:], axis=mybir.AxisListType.X)
                    eqr = wp.tile([128, 160], F32, tag="eqr", bufs=2)
                    nc.vector.tensor_scalar(eqr[:], iota160f[:], rank[:], None, OP.is_equal)
                    nc.tensor.matmul(ps_row[:], iota_pvf[:, pc:pc + 1], eqr[:],
                                     start=(pc == 0), stop=(pc == 3))
                psel_row = pp.tile([1, 160], F32, tag=f"pselr{cl}")
                nc.vector.tensor_copy(psel_row[:], ps_row[:])
                psel_rows[cl] = psel_row

            # unpack qkv to head-sharded layouts
            for src in range(NCORES):
                nc.sync.dma_start(qT[:, src * TOKS:(src + 1) * TOKS],
                                  qkv_recv.ap()[src, 0:128, :])
                kvflat = (qkv_recv.ap()[src, 128:384, :]
                          .rearrange("r c -> (r c)")
                          .rearrange("(s p c) -> p s c", p=128, c=2 * CHC))
                nc.sync.dma_start(k_sb[:, src * 16:(src + 1) * 16, :], kvflat[:, :, 0:CHC])
                nc.sync.dma_start(v_sb[:, src * 16:(src + 1) * 16, :], kvflat[:, :, CHC:2 * CHC])

            # ================= phase 3b: one-hot matmul gathers ==============
            for src in range(K if KCUT >= 2 else 0):
                psB2 = wp.tile([128, KGW], F32, tag="psB2", bufs=1)
                nc.vector.memset(psB2[:, FULL:KGW], -1.0)
                for f8 in range(FC):
                    fr = int(clusters[src][f8])
                    ps_b = psM.tile([128, 512], F32, tag="med")
                    nc.tensor.matmul(ps_b[:, 0:160], onesf_row[:], psel_rows[src][:],
                                     start=True, stop=True)
                    nc.vector.tensor_scalar(psB2[:, f8 * TK:(f8 + 1) * TK],
                                            ps_b[:, 0:TK], float(fr * P), None, OP.add)
                for c in range(10):
                    psk = psM.tile([128, 512], F32, tag="med")
                    psv = psM.tile([128, 512], F32, tag="med")
                    tcs = []
                    for f8 in _win_frames(c):
                        fr = int(clusters[src][f8])
                        tcs.extend(fr * 4 + i for i in range(4))
                    for ti, tc_ in enumerate(tcs):
                        ohW = wp.tile([128, 128], BF16, tag="ohW", bufs=4,
                                      name=f"ohW{src}_{c}_{ti}")
                        nc.vector.tensor_scalar(ohW[:], psB2[:, c * 128:(c + 1) * 128],
                                                iota_tcf[:, tc_:tc_ + 1], None, OP.is_equal)
                        nc.tensor.matmul(psk[:, 0:128], k_sb[:, tc_, :], ohW[:],
                                         start=(ti == 0), stop=(ti == len(tcs) - 1))
                        nc.tensor.matmul(psv[:, 0:128], ohW[:], v_sb[:, tc_, :],
                                         start=(ti == 0), stop=(ti == len(tcs) - 1))
                    nc.vector.tensor_copy(kg[:, src, c * 128:(c + 1) * 128], psk[:, 0:128])
                    nc.vector.tensor_copy(vaug[:, src, c, 0:64], psv[:, 0:64])
                    nc.vector.tensor_copy(vaug[:, src, c, 65:129], psv[:, 64:CHC])
            nc.vector.memset(vaug[:, :, :, 64:65], 1.0)
            nc.vector.memset(vaug[:, :, :, 129:130], 1.0)

            # ================= phase 4: attention, output-half pipelined ======
            for half in range(2):
              for ci in range(K if KCUT >= 3 else 0):
                chunks = _chunks_for(ci)
                for qt in ((0, 1, 4, 5) if half == 0 else (2, 3, 6, 7)):
                    f_q = int(clusters[ci][qt])
                    qsl = slice(f_q * P, (f_q + 1) * P)
                    ps_av = [psV.tile([65, 512], F32, tag="av", name=f"psav{ci}_{qt}_{i}") for i in range(2)]
                    nchk = len(chunks)
                    for g, (src, c, rows) in enumerate(chunks):
                        ps_lg = psL.tile([128, 1024], F32, tag="lg", bufs=2,
                                         name=f"pslg{ci}_{qt}_{g}")
                        for h in range(2):
                            nc.tensor.matmul(
                                ps_lg[:, h * 512:(h + 1) * 512],
                                kg[h * 64:(h + 1) * 64, src, c * 128:(c + 1) * 128],
                                qT[h * 64:(h + 1) * 64, qsl],
                                start=True, stop=True,
                                tile_position=(h * 64, 0))
                        ew = ep.tile([128, 1024], BF16, tag="ew", bufs=3)
                        nc.scalar.activation(ew[:], ps_lg[:], AF.Exp, scale=SCALE)
                        for h in range(2):
                            nc.tensor.matmul(
                                ps_av[h][:],
                                vaug[0:rows, src, c, h * 65:(h + 1) * 65],
                                ew[0:rows, h * 512:(h + 1) * 512],
                                start=(g == 0), stop=(g == nchk - 1))
                    # normalize and ship straight to the AllToAll staging buffer
                    otile = ep.tile([128, 512], BF16, tag="ot")
                    for h in range(2):
                        rec = wp.tile([1, 512], F32, tag="rec", bufs=1)
                        nc.vector.reciprocal(rec[:], ps_av[h][64:65, :])
                        ps_bc2 = psM.tile([64, 512], F32, tag="med")
                        nc.tensor.matmul(ps_bc2[:], onesf_row[:, 0:64], rec[:],
                                         start=True, stop=True)
                        bc_sb = wp.tile([64, 512], F32, tag="bcsb", bufs=1)
                        nc.vector.tensor_copy(bc_sb[:], ps_bc2[:])
                        nc.vector.tensor_tensor(
                            otile[h * 64:(h + 1) * 64, :],
                            ps_av[h][0:64, :], bc_sb[:], OP.mult)
                    jcore = (f_q * P) // TOKS
                    toff = (f_q * P) % TOKS % (TOKS // 2)
                    nc.sync.dma_start(ag_in[half].ap()[jcore, :, toff:toff + 512], otile[:])
              coll("AllToAll", OP.bypass, [ag_in[half].ap().opt()], [ag_out[half].ap().opt()])

            # ================= phase 5: proj, half-pipelined =================
            wpj = pp.tile([128, 8, C], BF16, tag="vsb", name="wpj")
            for dst in range(NCORES):
                nc.sync.dma_start(
                    wpj[:, :, dst * 128:(dst + 1) * 128],
                    wg_out.ap()[dst, :, 896:1024].rearrange("(a p) c -> p a c", p=128))
            bpj_row = pp.tile([1, C], BF16, tag="bpj")
            bpj_f = wp.tile([1, C], F32, tag="qs2", bufs=1, name="bpj_f")
            nc.sync.dma_start(bpj_f[:], bproj.ap().rearrange("(a c) -> a c", a=1))
            nc.vector.tensor_copy(bpj_row[:], bpj_f[:])
            for half in range(2):
                atk2 = pp.tile([128, 8, TOKS // 2], BF16,
                               tag=("ksb" if half == 0 else "qT"), name=f"atk2_{half}")
                nc.sync.dma_start(atk2[:], ag_out[half].ap().rearrange("j p t -> p j t"))
                for mt in range(TOKS // 256 if KCUT >= 4 else 0):
                    gmt = half * (TOKS // 256) + mt
                    for ntile in range(2):
                        nsl = slice(ntile * 512, (ntile + 1) * 512)
                        ps = psM.tile([128, 512], F32, tag="med")
                        for cc in range(8):
                            nc.tensor.matmul(ps[:], atk2[:, cc, mt * 128:(mt + 1) * 128],
                                             wpj[:, cc, nsl], start=(cc == 0), stop=False)
                        nc.tensor.matmul(ps[:], ones_rowb[:], bpj_row[:, nsl],
                                         start=False, stop=True)
                        ot = wp.tile([128, 512], mybir.dt.float16, tag="otile", bufs=2)
                        nc.vector.tensor_copy(ot[:], ps[:])
                        nc.sync.dma_start(
                            out_ext.ap()[gmt * 128:(gmt + 1) * 128, nsl], ot[:])

    nc.finalize()
    return nc


def _host_prep(x, W_qkv, b_qkv, W_proj, b_proj, clusters, keyframes):
    bf = ml_dtypes.bfloat16
    x2 = np.ascontiguousarray(x.reshape(N, C))
    kf_tok = np.concatenate([np.arange(P, dtype=np.int64) + int(f) * P for f in keyframes])

    in_maps = []
    for core in range(NCORES):
        xs = x2[core * TOKS:(core + 1) * TOKS]                        # [2048, C]
        xsT = np.ascontiguousarray(xs.T.astype(bf))
        kf_slice = kf_tok[core * KFC:(core + 1) * KFC]
        xkf = x2[kf_slice]                                            # [256, C] f32
        xkf_h = xkf.astype(bf)
        xkf_l = (xkf - xkf_h.astype(np.float32)).astype(bf)

        h0 = core * HC
        qcols = np.arange(h0 * D, (h0 + HC) * D)
        wq = W_qkv[:, qcols]
        wk = W_qkv[:, C + qcols]
        wv = W_qkv[:, 2 * C + qcols]
        wqk_hi = np.concatenate([wq, wk], axis=1).astype(bf).astype(np.float32)
        wqk_lo = np.concatenate([wq, wk], axis=1) - wqk_hi
        wcat = np.concatenate([
            wq, wk, wv,
            wqk_hi[:, 0:CHC], wqk_hi[:, CHC:2 * CHC],
            wqk_lo[:, 0:CHC], wqk_lo[:, CHC:2 * CHC],
            W_proj[:, core * CHC:(core + 1) * CHC],
        ], axis=1)                                                    # [C, 1024]
        in_maps.append({
            "xsT": xsT,
            "xkfT_h": np.ascontiguousarray(xkf_h.T),
            "xkfT_l": np.ascontiguousarray(xkf_l.T),
            "wcat": np.ascontiguousarray(wcat.astype(bf)),
            "bqkv": np.ascontiguousarray(b_qkv.astype(np.float32)),
            "bproj": np.ascontiguousarray(b_proj.astype(np.float32)),
        })
    return in_maps


def kernel(x, W_qkv, b_qkv, W_proj, b_proj, clusters, keyframes, **run_kwargs):
    x = np.asarray(x, dtype=np.float32)
    W_qkv = np.asarray(W_qkv, dtype=np.float32)
    b_qkv = np.asarray(b_qkv, dtype=np.float32)
    W_proj = np.asarray(W_proj, dtype=np.float32)
    b_proj = np.asarray(b_proj, dtype=np.float32)
    clusters = np.asarray(clusters, dtype=np.int32)
    keyframes = np.asarray(keyframes, dtype=np.int32)

    key = (clusters.tobytes(), keyframes.tobytes(), os.environ.get("KSTUB"))
    if _CACHE.get("key") != key:
        _CACHE["nc"] = build_nc(clusters, keyframes)
        _CACHE["key"] = key
    nc = _CACHE["nc"]

    in_maps = _host_prep(x, W_qkv, b_qkv, W_proj, b_proj, clusters, keyframes)
    res = run_bass_kernel_spmd(nc, in_maps, core_ids=list(range(NCORES)), **run_kwargs)
    _CACHE["last_result"] = res
    outs = res.results
    full = np.concatenate([np.asarray(outs[c]["out"], dtype=np.float32) for c in range(NCORES)], axis=0)
    return full.reshape(1, N, C)


def bench(x, W_qkv, b_qkv, W_proj, b_proj, clusters, keyframes, iters=10, reps=5):
    """Steady-state on-device timing: times the best of `reps` calls."""
    import time
    import jax
    from jax.sharding import Mesh, PartitionSpec
    from jax.experimental.shard_map import shard_map
    from concourse import bass2jax
    from concourse.bass2jax import _bass_exec_p
    import concourse.mybir as _mb

    clusters = np.asarray(clusters, dtype=np.int32)
    keyframes = np.asarray(keyframes, dtype=np.int32)
    key = (clusters.tobytes(), keyframes.tobytes(), os.environ.get("KSTUB"))
    if _CACHE.get("key") != key:
        _CACHE["nc"] = build_nc(clusters, keyframes)
        _CACHE["key"] = key
    nc = _CACHE["nc"]
    bass2jax.install_neuronx_cc_hook()

    in_maps = _host_prep(np.asarray(x, np.float32), np.asarray(W_qkv, np.float32),
                         np.asarray(b_qkv, np.float32), np.asarray(W_proj, np.float32),
                         np.asarray(b_proj, np.float32), clusters, keyframes)

    in_names, out_names, out_avals, zero_outs = [], [], [], []
    partition_name = nc.partition_id_tensor.name if nc.partition_id_tensor else None
    for alloc in nc.m.functions[0].allocations:
        if not isinstance(alloc, _mb.MemoryLocationSet):
            continue
        name = alloc.memorylocations[0].name
        if alloc.kind == "ExternalInput":
            if name != partition_name:
                in_names.append(name)
        elif alloc.kind == "ExternalOutput":
            out_names.append(name)
            shape = tuple(alloc.tensor_shape)
            dtype = _mb.dt.np(alloc.dtype)
            out_avals.append(jax.core.ShapedArray(shape, dtype))
            zero_outs.append(np.zeros(shape, dtype))
    n_params = len(in_names)
    all_in_names = list(in_names) + list(out_names)
    if partition_name is not None:
        all_in_names.append(partition_name)

    def _body(*args):
        ops = list(args)
        if partition_name is not None:
            ops = ops + [bass2jax.partition_id_tensor()]
        outs = _bass_exec_p.bind(
            *ops,
            out_avals=tuple(out_avals),
            in_names=tuple(all_in_names),
            out_names=tuple(out_names),
            lowering_input_output_aliases=(),
            sim_require_finite=True,
            sim_require_nnan=True,
            nc=nc,
        )
        return tuple(outs)

    devices = jax.devices()[:NCORES]
    mesh = Mesh(np.asarray(devices), ("core",))
    in_specs = (PartitionSpec("core"),) * (n_params + len(out_names))
    out_specs = (PartitionSpec("core"),) * len(out_names)
    f = jax.jit(shard_map(_body, mesh=mesh, in_specs=in_specs,
                          out_specs=out_specs, check_rep=False))
    concat_in = [np.concatenate([np.asarray(in_maps[c][n]) for c in range(NCORES)], axis=0)
                 for n in in_names]
    concat_zeros = [np.zeros((NCORES * z.shape[0], *z.shape[1:]), z.dtype) for z in zero_outs]
    args = [jax.device_put(a) for a in concat_in + concat_zeros]
    o = f(*args)
    jax.block_until_ready(o)
    times = []
    for _ in range(max(reps, 30)):
        t0 = time.perf_counter()
        o = f(*args)
        jax.block_until_ready(o)
        times.append(time.perf_counter() - t0)
    times.sort()
    return times[0] * 1e9, times


def bench_floor(reps=30):
    """Dispatch-floor: time a trivial 8-core NEFF (one 64KB copy)."""
    import time
    import jax
    from jax.sharding import Mesh, PartitionSpec
    from jax.experimental.shard_map import shard_map
    from concourse import bass2jax
    from concourse.bass2jax import _bass_exec_p
    import concourse.bacc as _bacc
    import concourse.tile as _tile

    if "floor_nc" not in _CACHE:
        nc = _bacc.Bacc(None, target_bir_lowering=False, debug=False)
        a = nc.dram_tensor("a", [128, 128], F32, kind="ExternalInput")
        b = nc.dram_tensor("b", [128, 128], F32, kind="ExternalOutput")
        with _tile.TileContext(nc) as tc:
            with tc.tile_pool(name="p", bufs=1) as p:
                t = p.tile([128, 128], F32)
                nc.sync.dma_start(t[:], a.ap())
                nc.sync.dma_start(b.ap(), t[:])
        nc.finalize()
        _CACHE["floor_nc"] = nc
    nc = _CACHE["floor_nc"]
    bass2jax.install_neuronx_cc_hook()
    partition_name = nc.partition_id_tensor.name if nc.partition_id_tensor else None
    in_names = ["a", "b"]
    if partition_name is not None:
        in_names.append(partition_name)
    out_avals = (jax.core.ShapedArray((128, 128), np.float32),)

    def _body(*args):
        ops = list(args)
        if partition_name is not None:
            ops = ops + [bass2jax.partition_id_tensor()]
        return tuple(_bass_exec_p.bind(
            *ops, out_avals=out_avals, in_names=tuple(in_names),
            out_names=("b",), lowering_input_output_aliases=(),
            sim_require_finite=True, sim_require_nnan=True, nc=nc))

    devices = jax.devices()[:NCORES]
    mesh = Mesh(np.asarray(devices), ("core",))
    f = jax.jit(shard_map(_body, mesh=mesh,
                          in_specs=(PartitionSpec("core"),) * 2,
                          out_specs=(PartitionSpec("core"),), check_rep=False))
    a = jax.device_put(np.zeros((NCORES * 128, 128), np.float32))
    z = jax.device_put(np.zeros((NCORES * 128, 128), np.float32))
    o = f(a, z); jax.block_until_ready(o)
    times = []
    for _ in range(reps):
        t0 = time.perf_counter()
        o = f(a, z)
        jax.block_until_ready(o)
        times.append(time.perf_counter() - t0)
    times.sort()
    return times[0] * 1e9

